# revision 3
# baseline (speedup 1.0000x reference)
"""Trainium2 Bass kernel for nn_MeanAggregator (segment mean + time features).

Hybrid gather: the kernel is GpSimd-SEQ-bound (994ns fixed SWDGE cost per
indirect DMA instruction, 128 rows each), while the DMA engines sit 84%
idle. So node slots k<6 use the [P,1] indirect-DMA path (600 instructions,
segment-aligned tiles, DVE adds), and slots k in 6..9 (50k rows/core) are
offloaded to bulk dma_gather (1024 rows per instruction, int16 window
bucketing) + SBUF parity dma_scatter_add, interleaved into the indirect
stream so their transfers hide under the indirect instructions' SEQ time.
A final merge adds the parity accumulators into the per-slot partial sums
and scales by 1/10.

HW constraints baked in (probed): bulk SWDGE instructions are limited to
1024 rows (2048 wedges the ring); dma_scatter_add loses duplicate-target
updates within one instruction, so cells are cut at (k, window) bucket
boundaries where targets are distinct; multi-index indirect offsets ([P,k],
k>1) gather garbage, so the indirect path stays at 128 rows/instruction.

Device emits only the active embed means [2500, 5, 256] f32; time features
(cos LUT over the int time values) and the constant pad half are assembled
host-side.
"""

import os
import sys

import numpy as np

sys.path.insert(0, "/opt/trn_rl_repo")

from contextlib import ExitStack

import concourse.bass as bass
import concourse.tile as tile
from concourse import bacc, mybir
from concourse._compat import with_exitstack
from concourse.bass_utils import run_bass_kernel_spmd

N_CORES = 8
NUM_ENTITIES = 200000
H = 256
T = 32
SEQ_LEN = 10
N_EXAMPLES = 20000
SEGS_PER_EX = 5
NODES_PER_SEG = 10
EX_PER_CORE = N_EXAMPLES // N_CORES  # 2500
P = 128
NBLK = (EX_PER_CORE + P - 1) // P  # 20
PAD_TIME = 1000000.0

NOFF = 4  # node slots offloaded to the bulk path (k in 10-NOFF .. 9)
KIND = NODES_PER_SEG - NOFF  # 6 slots on the indirect path
WINDOW = 32768
N_CELL = 1024
TOTAL_SLOTS = NBLK * SEGS_PER_EX  # 100
DUMMY_SP = TOTAL_SLOTS * P
NGROUPS = (TOTAL_SLOTS >> 1) + 1  # 51

_CACHE = {}


def _plan_bulk(flat_s):
    """Window-bucketed cells for the offloaded node slots of every core.

    Returns (cells [(w, n)], per-core gidx/sidx arrays [ncells, 128, n/16])
    with identical static shapes across cores. Within a (k, w) bucket all
    scatter targets are distinct segments.
    """
    n_windows = (NUM_ENTITIES + WINDOW - 1) // WINDOW
    fs = flat_s.reshape(N_EXAMPLES, SEGS_PER_EX, NODES_PER_SEG)
    streams = []  # per core: (w, rel, sp, k) sorted
    for c in range(N_CORES):
        e0 = c * EX_PER_CORE
        g = fs[e0 : e0 + EX_PER_CORE, :, KIND:].astype(np.int64)  # [EXC,5,NOFF]
        e_local = np.arange(EX_PER_CORE)[:, None, None]
        j = np.arange(SEGS_PER_EX)[None, :, None]
        slot = (e_local // P) * SEGS_PER_EX + j
        sp = (slot * P + (e_local % P)) * np.ones((1, 1, NOFF), np.int64)
        k = np.broadcast_to(np.arange(NOFF)[None, None, :], g.shape)
        g, sp, k = g.ravel(), sp.ravel().astype(np.int16), k.ravel()
        w = (g // WINDOW).astype(np.int32)
        rel = (g % WINDOW).astype(np.int16)
        order = np.lexsort((sp, w, k))
        streams.append((w[order], rel[order], sp[order], k[order]))

    n_buckets = NOFF * n_windows
    bc = np.zeros((N_CORES, n_buckets), np.int64)
    for c in range(N_CORES):
        w, rel, sp, k = streams[c]
        bc[c] = np.bincount(k * n_windows + w, minlength=n_buckets)
    lb = [int(-(-int(bc[:, b].max()) // P) * P) for b in range(n_buckets)]
    cells = []
    for b in range(n_buckets):
        off = 0
        while off < lb[b]:
            n = min(N_CELL, lb[b] - off)
            cells.append((b % n_windows, n))
            off += n
    ncw = N_CELL // 16
    gidx_all, sidx_all = [], []
    for c in range(N_CORES):
        w, rel, sp, k = streams[c]
        bucket = k * n_windows + w
        gidx = np.zeros((len(cells), P, ncw), np.int16)
        sidx = np.full((len(cells), P, ncw), DUMMY_SP, np.int16)
        ci = 0
        for b in range(n_buckets):
            sel = bucket == b
            r, s = rel[sel], sp[sel]
            pad = lb[b] - len(r)
            if pad:
                r = np.concatenate([r, np.zeros(pad, np.int16)])
                s = np.concatenate([s, np.full(pad, DUMMY_SP, np.int16)])
            off = 0
            while off < lb[b]:
                n = cells[ci][1]
                gw = r[off : off + n].reshape(n // 16, 16).T
                sw = s[off : off + n].reshape(n // 16, 16).T
                gidx[ci, :, : n // 16] = np.tile(gw, (8, 1))
                sidx[ci, :, : n // 16] = np.tile(sw, (8, 1))
                off += n
                ci += 1
        gidx_all.append(gidx)
        sidx_all.append(sidx)
    win_len = [min(WINDOW, NUM_ENTITIES - w * WINDOW) for w in range(n_windows)]
    return cells, win_len, gidx_all, sidx_all


@with_exitstack
def _emit(ctx: ExitStack, tc, table, idx, gidx, sidx, out, cells, win_len):
    nc = tc.nc
    f32 = mybir.dt.float32
    bf16 = mybir.dt.bfloat16
    i16 = mybir.dt.int16

    g_pool = ctx.enter_context(tc.tile_pool(name="g", bufs=66))
    io_pool = ctx.enter_context(tc.tile_pool(name="io", bufs=4))
    bi_pool = ctx.enter_context(tc.tile_pool(name="bi", bufs=6))
    bg_pool = ctx.enter_context(tc.tile_pool(name="bg", bufs=3))
    out_pool = ctx.enter_context(tc.tile_pool(name="outp", bufs=4))
    per_pool = ctx.enter_context(tc.tile_pool(name="per", bufs=1))

    # persistent accumulators: per-slot partial sums (indirect path) and
    # the bulk path's parity pair
    acc8 = per_pool.tile([P, TOTAL_SLOTS, H], bf16)
    pa = per_pool.tile([P, NGROUPS, H], bf16)
    pb = per_pool.tile([P, NGROUPS, H], bf16)
    nc.vector.memset(pa[:], 0)
    nc.vector.memset(pb[:], 0)

    # bulk instruction stream, interleaved into the block loop
    bulk = []  # ("G", ci) / ("S", ci, gt, si)
    for ci in range(len(cells)):
        bulk.append(("G", ci))
    per_block = -(-len(bulk) * 2 // NBLK)  # G+S entries per block, ceil
    pend = []

    def emit_bulk_one():
        if pend and (pend[0][0] != "G_wait" or True):
            pass
        if bulk and (not pend or len(pend) >= 2):
            pass
        # emit next: prefer scatter for the oldest gathered cell once a
        # newer gather has been issued; else next gather
        if len(pend) >= 2 or (not bulk and pend):
            ci, gt, si = pend.pop(0)
            w, n = cells[ci]
            nc.gpsimd.dma_scatter_add(
                pa[:], gt[:], si[:], n, n, H,
                sbuf_tokens_per_rank=P, parity_reg=0, out_ap_other=pb[:],
                queue_num=0,
            )
            return True
        if bulk:
            _, ci = bulk.pop(0)
            w, n = cells[ci]
            gi = bi_pool.tile([P, n // 16], i16)
            nc.sync.dma_start(out=gi[:], in_=gidx[ci, :, 0 : n // 16])
            si = bi_pool.tile([P, n // 16], i16)
            nc.sync.dma_start(out=si[:], in_=sidx[ci, :, 0 : n // 16])
            gt = bg_pool.tile([P, n // P, H], bf16)
            wb = w * WINDOW
            nc.gpsimd.dma_gather(
                gt[:], table[wb : wb + win_len[w]], gi[:], n, n, H,
                queue_num=0,
            )
            pend.append((ci, gt, si))
            return True
        return False

    for b in range(NBLK):
        npar = min(P, EX_PER_CORE - b * P)
        idx_t = io_pool.tile([P, SEGS_PER_EX * KIND], mybir.dt.int32)
        nc.sync.dma_start(out=idx_t[:npar], in_=idx[b, :npar])

        for j in range(SEGS_PER_EX):
            gs = []
            for k in range(KIND):
                c = j * KIND + k
                g = g_pool.tile([P, H], bf16)
                nc.gpsimd.indirect_dma_start(
                    out=g[:npar],
                    out_offset=None,
                    in_=table,
                    in_offset=bass.IndirectOffsetOnAxis(
                        ap=idx_t[:npar, c : c + 1], axis=0
                    ),
                )
                gs.append(g)
            emit_bulk_one()
            slot = b * SEGS_PER_EX + j
            nc.vector.tensor_tensor(
                out=acc8[:npar, slot], in0=gs[0][:npar], in1=gs[1][:npar],
                op=mybir.AluOpType.add,
            )
            for k in range(2, KIND):
                nc.vector.tensor_tensor(
                    out=acc8[:npar, slot], in0=acc8[:npar, slot],
                    in1=gs[k][:npar], op=mybir.AluOpType.add,
                )
        emit_bulk_one()
    while bulk or pend:
        if not emit_bulk_one():
            break

    # merge: acc8 + parity accumulators, scale 1/10, write out
    for b in range(NBLK):
        npar = min(P, EX_PER_CORE - b * P)
        out_t = out_pool.tile([P, SEGS_PER_EX, H], f32)
        for j in range(SEGS_PER_EX):
            slot = b * SEGS_PER_EX + j
            par = pa if slot % 2 == 0 else pb
            nc.vector.tensor_tensor(
                out=acc8[:npar, slot], in0=acc8[:npar, slot],
                in1=par[:npar, slot >> 1], op=mybir.AluOpType.add,
            )
            nc.scalar.mul(
                out_t[:npar, j, 0:H], acc8[:npar, slot], 1.0 / NODES_PER_SEG
            )
        rows = slice(b * P, b * P + npar)
        nc.sync.dma_start(out=out[rows, :, :], in_=out_t[:npar])


def _build_nc(cells, win_len):
    nc = bacc.Bacc(
        "TRN2",
        target_bir_lowering=False,
        debug=False,
        enable_asserts=False,
        num_devices=N_CORES,
        num_swdge_queues=1,
        dynamic_dma_scratch_size=32768,
    )
    f32 = mybir.dt.float32
    table = nc.dram_tensor(
        "table", [NUM_ENTITIES, H], mybir.dt.bfloat16, kind="ExternalInput"
    ).ap()
    idx = nc.dram_tensor(
        "idx", [NBLK, P, SEGS_PER_EX * KIND], mybir.dt.int32,
        kind="ExternalInput",
    ).ap()
    gidx = nc.dram_tensor(
        "gidx", [len(cells), P, N_CELL // 16], mybir.dt.int16,
        kind="ExternalInput",
    ).ap()
    sidx = nc.dram_tensor(
        "sidx", [len(cells), P, N_CELL // 16], mybir.dt.int16,
        kind="ExternalInput",
    ).ap()
    out = nc.dram_tensor(
        "out", [EX_PER_CORE, SEGS_PER_EX, H], f32, kind="ExternalOutput"
    ).ap()
    with tile.TileContext(nc) as tc:
        _emit(tc, table, idx, gidx, sidx, out, cells, win_len)
    nc.compile()
    return nc


def kernel(
    ent_embeds, t_w, t_b, flat_s, node_seg_ids, seg_example, seg_pos, time_vals
):
    import ml_dtypes

    ent_embeds = np.ascontiguousarray(
        np.asarray(ent_embeds, dtype=np.float32).astype(ml_dtypes.bfloat16)
    )
    t_w = np.asarray(t_w, dtype=np.float32)
    t_b = np.asarray(t_b, dtype=np.float32)
    flat_s = np.asarray(flat_s, dtype=np.int32)
    seg_example = np.asarray(seg_example, dtype=np.int32)
    seg_pos = np.asarray(seg_pos, dtype=np.int32)
    time_vals = np.asarray(time_vals, dtype=np.int32)

    cells, win_len, gidx_all, sidx_all = _plan_bulk(flat_s)
    key = tuple(cells)
    if _CACHE.get("key") != key:
        _CACHE["nc"] = _build_nc(cells, win_len)
        _CACHE["key"] = key
    nc = _CACHE["nc"]

    fs = flat_s.reshape(N_EXAMPLES, SEGS_PER_EX, NODES_PER_SEG)
    in_maps = []
    for c in range(N_CORES):
        e0 = c * EX_PER_CORE
        ind = fs[e0 : e0 + EX_PER_CORE, :, :KIND].reshape(
            EX_PER_CORE, SEGS_PER_EX * KIND
        )
        idx_host = np.zeros((NBLK * P, SEGS_PER_EX * KIND), np.int32)
        idx_host[:EX_PER_CORE] = ind
        in_maps.append(
            {
                "table": ent_embeds,
                "idx": idx_host.reshape(NBLK, P, SEGS_PER_EX * KIND),
                "gidx": gidx_all[c],
                "sidx": sidx_all[c],
            }
        )

    trace = os.environ.get("BASSKERNEL_TRACE", "0") == "1"
    kw = {}
    if trace:
        kw = dict(trace=True, tmpdir=os.environ.get("BASSKERNEL_TRACEDIR") or None)
    res = run_bass_kernel_spmd(nc, in_maps, core_ids=list(range(N_CORES)), **kw)
    if trace:
        _CACHE["last_results"] = res
        print(f"[kernel] exec_time_ns={res.exec_time_ns}", file=sys.stderr)

    out = np.empty((N_EXAMPLES, SEQ_LEN, H + T), np.float32)
    pad_vec = np.cos(np.float32(PAD_TIME) * t_w + t_b).astype(np.float32)
    out[:, :, H:] = pad_vec
    out[:, :, :H] = 0.0
    uvals, inv = np.unique(time_vals, return_inverse=True)
    lutv = np.cos(uvals[:, None].astype(np.float32) * t_w + t_b).astype(
        np.float32
    )
    out[seg_example, seg_pos, H:] = lutv[inv]
    for c in range(N_CORES):
        e0 = c * EX_PER_CORE
        out[e0 : e0 + EX_PER_CORE, :SEGS_PER_EX, :H] = np.asarray(
            res.results[c]["out"]
        )
    return out


# revision 4
# speedup vs baseline: 1.2992x; 1.2992x over previous
"""Trainium2 Bass kernel for nn_MeanAggregator (segment mean + time features).

Computation (see reference):
  out[e, p, 0:256]   = mean of 10 gathered ent_embeds rows of segment 5e+p   (p < 5)
  out[e, p, 256:288] = cos(t * t_w + t_b), t = time_vals[5e+p]               (p < 5)
  out[e, p, 0:256]   = 0                                                      (p >= 5)
  out[e, p, 256:288] = cos(1e6 * t_w + t_b)                                   (p >= 5)

Sharding: data-parallel over examples; core c owns examples [2500c, 2500(c+1)).
Device work per core: 125k-row bf16 gather ([P,1] indirect DMA per node slot;
the table is cast to bf16 host-side, halving gather bytes — output tolerance
2e-2 dwarfs bf16 rounding), DVE tensor adds for the 10-row segment sums
(bf16, 2x DVE rate), ScalarE scale-to-f32 into the output tile. The gather
tile pool is sized for full double-buffering across example blocks.

The device emits ONLY the active embed means [2500, 5, 256] f32 (12.8MB/core).
Time features (a 300-entry cos LUT over the int time values) and the constant
pad half are assembled host-side — they were host-precomputed and merely
shuttled through the device before, costing 16MB/core of extra DMA.

Note: bulk `dma_gather` (SWDGE extended instr) amortizes the 994ns/instr
SWDGE overhead 8x, but the re-association it forces (int16 window bucketing +
dma_scatter_add) costs ~90ns of DMA-engine time per scattered 512B write and
serializes on the ~2048-slot descriptor ring: measured 1.63ms end-to-end vs
1.45ms for this layout. Multi-index indirect offset APs ([P,k], k>1) gather
garbage on HW (probed); [P,1] is the real limit.
"""

import math
import os
import sys

import numpy as np

sys.path.insert(0, "/opt/trn_rl_repo")

from contextlib import ExitStack

import concourse.bass as bass
import concourse.tile as tile
from concourse import bacc, mybir
from concourse._compat import with_exitstack
from concourse.bass_utils import run_bass_kernel_spmd

# Problem constants (hardcoded; kernel.py must be self-contained).
N_CORES = 8
NUM_ENTITIES = 200000
H = 256
T = 32
SEQ_LEN = 10
N_EXAMPLES = 20000
SEGS_PER_EX = 5
NODES_PER_SEG = 10
EX_PER_CORE = N_EXAMPLES // N_CORES  # 2500
P = 128
NBLK = (EX_PER_CORE + P - 1) // P  # 20
PAD_TIME = 1000000.0

_CACHE = {}


@with_exitstack
def _emit(ctx: ExitStack, tc, table, idx, out):
    nc = tc.nc
    f32 = mybir.dt.float32

    # 50 gather tiles per example-block; 110 bufs give full double-buffering
    # (block b+1's gathers never wait on block b's adds releasing tiles).
    g_pool = ctx.enter_context(tc.tile_pool(name="g", bufs=110))
    io_pool = ctx.enter_context(tc.tile_pool(name="io", bufs=4))
    out_pool = ctx.enter_context(tc.tile_pool(name="outp", bufs=4))
    acc_pool = ctx.enter_context(tc.tile_pool(name="acc", bufs=12))

    for b in range(NBLK):
        npar = min(P, EX_PER_CORE - b * P)
        idx_t = io_pool.tile([P, SEGS_PER_EX * NODES_PER_SEG], mybir.dt.int32)
        nc.sync.dma_start(out=idx_t[:npar], in_=idx[b, :npar])
        out_t = out_pool.tile([P, SEGS_PER_EX, H], f32)

        gsj = []
        for j in range(SEGS_PER_EX):
            gs = []
            for k in range(NODES_PER_SEG):
                c = j * NODES_PER_SEG + k
                g = g_pool.tile([P, H], mybir.dt.bfloat16)
                # HW indirect DMA only honors [P, 1] offset APs (one index
                # per partition); multi-index offsets gather garbage.
                nc.gpsimd.indirect_dma_start(
                    out=g[:npar],
                    out_offset=None,
                    in_=table,
                    in_offset=bass.IndirectOffsetOnAxis(
                        ap=idx_t[:npar, c : c + 1], axis=0
                    ),
                )
                gs.append(g)
            gsj.append(gs)
        for j in range(SEGS_PER_EX):
            gs = gsj[j]
            acc = acc_pool.tile([P, H], mybir.dt.bfloat16)
            nc.vector.tensor_tensor(
                out=acc[:npar], in0=gs[0][:npar], in1=gs[1][:npar],
                op=mybir.AluOpType.add,
            )
            for k in range(2, NODES_PER_SEG):
                nc.vector.tensor_tensor(
                    out=acc[:npar], in0=acc[:npar], in1=gs[k][:npar],
                    op=mybir.AluOpType.add,
                )
            nc.scalar.mul(out_t[:npar, j, 0:H], acc[:npar], 1.0 / NODES_PER_SEG)

        rows = slice(b * P, b * P + npar)
        nc.sync.dma_start(out=out[rows, :, :], in_=out_t[:npar])


def _build_nc():
    nc = bacc.Bacc(
        "TRN2",
        target_bir_lowering=False,
        debug=False,
        enable_asserts=False,
        num_devices=N_CORES,
    )
    f32 = mybir.dt.float32
    table = nc.dram_tensor(
        "table", [NUM_ENTITIES, H], mybir.dt.bfloat16, kind="ExternalInput"
    ).ap()
    idx = nc.dram_tensor(
        "idx", [NBLK, P, SEGS_PER_EX * NODES_PER_SEG], mybir.dt.int32,
        kind="ExternalInput",
    ).ap()
    out = nc.dram_tensor(
        "out", [EX_PER_CORE, SEGS_PER_EX, H], f32, kind="ExternalOutput"
    ).ap()
    with tile.TileContext(nc) as tc:
        _emit(tc, table, idx, out)
    nc.compile()
    return nc


def kernel(
    ent_embeds, t_w, t_b, flat_s, node_seg_ids, seg_example, seg_pos, time_vals
):
    import ml_dtypes

    ent_embeds = np.ascontiguousarray(
        np.asarray(ent_embeds, dtype=np.float32).astype(ml_dtypes.bfloat16)
    )
    t_w = np.asarray(t_w, dtype=np.float32)
    t_b = np.asarray(t_b, dtype=np.float32)
    flat_s = np.asarray(flat_s, dtype=np.int32)
    seg_example = np.asarray(seg_example, dtype=np.int32)
    seg_pos = np.asarray(seg_pos, dtype=np.int32)
    time_vals = np.asarray(time_vals, dtype=np.int32)

    if "nc" not in _CACHE:
        _CACHE["nc"] = _build_nc()
    nc = _CACHE["nc"]

    in_maps = []
    for c in range(N_CORES):
        e0 = c * EX_PER_CORE
        fs = flat_s[
            e0 * SEGS_PER_EX * NODES_PER_SEG : (e0 + EX_PER_CORE)
            * SEGS_PER_EX
            * NODES_PER_SEG
        ].reshape(EX_PER_CORE, SEGS_PER_EX * NODES_PER_SEG)
        idx_host = np.zeros((NBLK * P, SEGS_PER_EX * NODES_PER_SEG), np.int32)
        idx_host[:EX_PER_CORE] = fs
        in_maps.append(
            {
                "table": ent_embeds,
                "idx": idx_host.reshape(NBLK, P, SEGS_PER_EX * NODES_PER_SEG),
            }
        )

    trace = os.environ.get("BASSKERNEL_TRACE", "0") == "1"
    kw = {}
    if trace:
        kw = dict(trace=True, tmpdir=os.environ.get("BASSKERNEL_TRACEDIR") or None)
    res = run_bass_kernel_spmd(nc, in_maps, core_ids=list(range(N_CORES)), **kw)
    if trace:
        _CACHE["last_results"] = res
        print(f"[kernel] exec_time_ns={res.exec_time_ns}", file=sys.stderr)

    # Host assembly: time features from a cos LUT over the distinct int t
    # values; pad half is constant; device shards fill the active embeds.
    out = np.empty((N_EXAMPLES, SEQ_LEN, H + T), np.float32)
    pad_vec = np.cos(np.float32(PAD_TIME) * t_w + t_b).astype(np.float32)
    out[:, :, H:] = pad_vec
    out[:, :, :H] = 0.0
    uvals, inv = np.unique(time_vals, return_inverse=True)
    lutv = np.cos(uvals[:, None].astype(np.float32) * t_w + t_b).astype(
        np.float32
    )
    out[seg_example, seg_pos, H:] = lutv[inv]
    for c in range(N_CORES):
        e0 = c * EX_PER_CORE
        out[e0 : e0 + EX_PER_CORE, :SEGS_PER_EX, :H] = np.asarray(
            res.results[c]["out"]
        )
    return out


# revision 5
# speedup vs baseline: 1.5028x; 1.1567x over previous
"""Trainium2 Bass kernel for nn_MeanAggregator (segment mean + time features).

Computation (see reference):
  out[e, p, 0:256]   = mean of 10 gathered ent_embeds rows of segment 5e+p   (p < 5)
  out[e, p, 256:288] = cos(t * t_w + t_b), t = time_vals[5e+p]               (p < 5)
  out[e, p, 0:256]   = 0                                                      (p >= 5)
  out[e, p, 256:288] = cos(1e6 * t_w + t_b)                                   (p >= 5)

Sharding: data-parallel over examples; core c owns examples [2500c, 2500(c+1)).
Device work per core: 125k-row bf16 gather ([P,1] indirect DMA per node slot;
the table is cast to bf16 host-side, halving gather bytes — output tolerance
2e-2 dwarfs bf16 rounding), DVE tensor adds for the 10-row segment sums
(bf16, 2x DVE rate), ScalarE scale-to-f32 into the output tile. The gather
tile pool is sized for full double-buffering across example blocks.

The device emits ONLY the active embed means [2500, 5, 256] f32 (12.8MB/core).
Time features (a 300-entry cos LUT over the int time values) and the constant
pad half are assembled host-side — they were host-precomputed and merely
shuttled through the device before, costing 16MB/core of extra DMA.

Note: bulk `dma_gather` (SWDGE extended instr) amortizes the 994ns/instr
SWDGE overhead 8x, but the re-association it forces (int16 window bucketing +
dma_scatter_add) costs ~90ns of DMA-engine time per scattered 512B write and
serializes on the ~2048-slot descriptor ring: measured 1.63ms end-to-end vs
1.45ms for this layout. Multi-index indirect offset APs ([P,k], k>1) gather
garbage on HW (probed); [P,1] is the real limit.
"""

import math
import os
import sys

import numpy as np

sys.path.insert(0, "/opt/trn_rl_repo")

from contextlib import ExitStack

import concourse.bass as bass
import concourse.tile as tile
from concourse import bacc, mybir
from concourse._compat import with_exitstack
from concourse.bass_utils import run_bass_kernel_spmd

# Problem constants (hardcoded; kernel.py must be self-contained).
N_CORES = 8
NUM_ENTITIES = 200000
H = 256
T = 32
SEQ_LEN = 10
N_EXAMPLES = 20000
SEGS_PER_EX = 5
NODES_PER_SEG = 10
EX_PER_CORE = N_EXAMPLES // N_CORES  # 2500
P = 128
NBLK = (EX_PER_CORE + P - 1) // P  # 20
PAD_TIME = 1000000.0

_CACHE = {}


@with_exitstack
def _emit(ctx: ExitStack, tc, table, idx, out):
    nc = tc.nc
    f32 = mybir.dt.float32

    # 50 gather tiles per example-block; 160 bufs give >3 blocks of
    # double-buffering (block b+1's gathers never wait on block b's adds
    # releasing tiles). SBUF freed by dropping the tf/pad path pays for it.
    g_pool = ctx.enter_context(tc.tile_pool(name="g", bufs=160))
    io_pool = ctx.enter_context(tc.tile_pool(name="io", bufs=NBLK))
    out_pool = ctx.enter_context(tc.tile_pool(name="outp", bufs=6))
    acc_pool = ctx.enter_context(tc.tile_pool(name="acc", bufs=20))

    # prefetch every block's index tile upfront (4KB/partition total) so
    # no gather ever waits on an index load mid-stream
    idx_tiles = []
    for b in range(NBLK):
        npar = min(P, EX_PER_CORE - b * P)
        it = io_pool.tile([P, SEGS_PER_EX * NODES_PER_SEG], mybir.dt.int32)
        nc.sync.dma_start(out=it[:npar], in_=idx[b, :npar])
        idx_tiles.append(it)

    for b in range(NBLK):
        npar = min(P, EX_PER_CORE - b * P)
        idx_t = idx_tiles[b]
        out_t = out_pool.tile([P, SEGS_PER_EX, H], f32)

        gsj = []
        for j in range(SEGS_PER_EX):
            gs = []
            for k in range(NODES_PER_SEG):
                c = j * NODES_PER_SEG + k
                g = g_pool.tile([P, H], mybir.dt.bfloat16)
                # HW indirect DMA only honors [P, 1] offset APs (one index
                # per partition); multi-index offsets gather garbage.
                nc.gpsimd.indirect_dma_start(
                    out=g[:npar],
                    out_offset=None,
                    in_=table,
                    in_offset=bass.IndirectOffsetOnAxis(
                        ap=idx_t[:npar, c : c + 1], axis=0
                    ),
                )
                gs.append(g)
            gsj.append(gs)
        for j in range(SEGS_PER_EX):
            gs = gsj[j]
            acc = acc_pool.tile([P, H], mybir.dt.bfloat16)
            nc.vector.tensor_tensor(
                out=acc[:npar], in0=gs[0][:npar], in1=gs[1][:npar],
                op=mybir.AluOpType.add,
            )
            for k in range(2, NODES_PER_SEG):
                nc.vector.tensor_tensor(
                    out=acc[:npar], in0=acc[:npar], in1=gs[k][:npar],
                    op=mybir.AluOpType.add,
                )
            nc.scalar.mul(out_t[:npar, j, 0:H], acc[:npar], 1.0 / NODES_PER_SEG)

        rows = slice(b * P, b * P + npar)
        nc.sync.dma_start(out=out[rows, :, :], in_=out_t[:npar])


def _build_nc():
    nc = bacc.Bacc(
        "TRN2",
        target_bir_lowering=False,
        debug=False,
        enable_asserts=False,
        num_devices=N_CORES,
    )
    f32 = mybir.dt.float32
    table = nc.dram_tensor(
        "table", [NUM_ENTITIES, H], mybir.dt.bfloat16, kind="ExternalInput"
    ).ap()
    idx = nc.dram_tensor(
        "idx", [NBLK, P, SEGS_PER_EX * NODES_PER_SEG], mybir.dt.int32,
        kind="ExternalInput",
    ).ap()
    out = nc.dram_tensor(
        "out", [EX_PER_CORE, SEGS_PER_EX, H], f32, kind="ExternalOutput"
    ).ap()
    with tile.TileContext(nc) as tc:
        _emit(tc, table, idx, out)
    nc.compile()
    return nc


def kernel(
    ent_embeds, t_w, t_b, flat_s, node_seg_ids, seg_example, seg_pos, time_vals
):
    import ml_dtypes

    ent_embeds = np.ascontiguousarray(
        np.asarray(ent_embeds, dtype=np.float32).astype(ml_dtypes.bfloat16)
    )
    t_w = np.asarray(t_w, dtype=np.float32)
    t_b = np.asarray(t_b, dtype=np.float32)
    flat_s = np.asarray(flat_s, dtype=np.int32)
    seg_example = np.asarray(seg_example, dtype=np.int32)
    seg_pos = np.asarray(seg_pos, dtype=np.int32)
    time_vals = np.asarray(time_vals, dtype=np.int32)

    if "nc" not in _CACHE:
        _CACHE["nc"] = _build_nc()
    nc = _CACHE["nc"]

    in_maps = []
    for c in range(N_CORES):
        e0 = c * EX_PER_CORE
        fs = flat_s[
            e0 * SEGS_PER_EX * NODES_PER_SEG : (e0 + EX_PER_CORE)
            * SEGS_PER_EX
            * NODES_PER_SEG
        ].reshape(EX_PER_CORE, SEGS_PER_EX * NODES_PER_SEG)
        idx_host = np.zeros((NBLK * P, SEGS_PER_EX * NODES_PER_SEG), np.int32)
        idx_host[:EX_PER_CORE] = fs
        in_maps.append(
            {
                "table": ent_embeds,
                "idx": idx_host.reshape(NBLK, P, SEGS_PER_EX * NODES_PER_SEG),
            }
        )

    trace = os.environ.get("BASSKERNEL_TRACE", "0") == "1"
    kw = {}
    if trace:
        kw = dict(trace=True, tmpdir=os.environ.get("BASSKERNEL_TRACEDIR") or None)
    res = run_bass_kernel_spmd(nc, in_maps, core_ids=list(range(N_CORES)), **kw)
    if trace:
        _CACHE["last_results"] = res
        print(f"[kernel] exec_time_ns={res.exec_time_ns}", file=sys.stderr)

    # Host assembly: time features from a cos LUT over the distinct int t
    # values; pad half is constant; device shards fill the active embeds.
    out = np.empty((N_EXAMPLES, SEQ_LEN, H + T), np.float32)
    pad_vec = np.cos(np.float32(PAD_TIME) * t_w + t_b).astype(np.float32)
    out[:, :, H:] = pad_vec
    out[:, :, :H] = 0.0
    uvals, inv = np.unique(time_vals, return_inverse=True)
    lutv = np.cos(uvals[:, None].astype(np.float32) * t_w + t_b).astype(
        np.float32
    )
    out[seg_example, seg_pos, H:] = lutv[inv]
    for c in range(N_CORES):
        e0 = c * EX_PER_CORE
        out[e0 : e0 + EX_PER_CORE, :SEGS_PER_EX, :H] = np.asarray(
            res.results[c]["out"]
        )
    return out


# revision 7
# speedup vs baseline: 1.5109x; 1.0054x over previous
"""Trainium2 Bass kernel for nn_MeanAggregator (segment mean + time features).

Computation (see reference):
  out[e, p, 0:256]   = mean of 10 gathered ent_embeds rows of segment 5e+p   (p < 5)
  out[e, p, 256:288] = cos(t * t_w + t_b), t = time_vals[5e+p]               (p < 5)
  out[e, p, 0:256]   = 0                                                      (p >= 5)
  out[e, p, 256:288] = cos(1e6 * t_w + t_b)                                   (p >= 5)

Sharding: data-parallel over examples; core c owns examples [2500c, 2500(c+1)).
Device work per core: 125k-row bf16 gather ([P,1] indirect DMA per node slot;
the table is cast to bf16 host-side, halving gather bytes — output tolerance
2e-2 dwarfs bf16 rounding), DVE tensor adds for the 10-row segment sums
(bf16, 2x DVE rate), ScalarE scale-to-f32 into the output tile. The gather
tile pool is sized for full double-buffering across example blocks.

The device emits ONLY the active embed means [2500, 5, 256] f32 (12.8MB/core).
Time features (a 300-entry cos LUT over the int time values) and the constant
pad half are assembled host-side — they were host-precomputed and merely
shuttled through the device before, costing 16MB/core of extra DMA.

Note: bulk `dma_gather` (SWDGE extended instr) amortizes the 994ns/instr
SWDGE overhead 8x, but the re-association it forces (int16 window bucketing +
dma_scatter_add) costs ~90ns of DMA-engine time per scattered 512B write and
serializes on the ~2048-slot descriptor ring: measured 1.63ms end-to-end vs
1.45ms for this layout. Multi-index indirect offset APs ([P,k], k>1) gather
garbage on HW (probed); [P,1] is the real limit.
"""

import math
import os
import sys

import numpy as np

sys.path.insert(0, "/opt/trn_rl_repo")

from contextlib import ExitStack

import concourse.bass as bass
import concourse.tile as tile
from concourse import bacc, mybir
from concourse._compat import with_exitstack
from concourse.bass_utils import run_bass_kernel_spmd

# Problem constants (hardcoded; kernel.py must be self-contained).
N_CORES = 8
NUM_ENTITIES = 200000
H = 256
T = 32
SEQ_LEN = 10
N_EXAMPLES = 20000
SEGS_PER_EX = 5
NODES_PER_SEG = 10
EX_PER_CORE = N_EXAMPLES // N_CORES  # 2500
P = 128
NBLK = (EX_PER_CORE + P - 1) // P  # 20
PAD_TIME = 1000000.0

_CACHE = {}


@with_exitstack
def _emit(ctx: ExitStack, tc, table, idx, out):
    nc = tc.nc
    f32 = mybir.dt.float32

    # 50 gather tiles per example-block; 110 bufs give full double-buffering
    # (block b+1's gathers never wait on block b's adds releasing tiles).
    g_pool = ctx.enter_context(tc.tile_pool(name="g", bufs=110))
    io_pool = ctx.enter_context(tc.tile_pool(name="io", bufs=4))
    out_pool = ctx.enter_context(tc.tile_pool(name="outp", bufs=4))
    acc_pool = ctx.enter_context(tc.tile_pool(name="acc", bufs=12))

    for b in range(NBLK):
        npar = min(P, EX_PER_CORE - b * P)
        idx_t = io_pool.tile([P, SEGS_PER_EX * NODES_PER_SEG], mybir.dt.int32)
        nc.sync.dma_start(out=idx_t[:npar], in_=idx[b, :npar])
        out_t = out_pool.tile([P, SEGS_PER_EX, H], f32)

        gsj = []
        for j in range(SEGS_PER_EX):
            gs = []
            for k in range(NODES_PER_SEG):
                c = j * NODES_PER_SEG + k
                g = g_pool.tile([P, H], mybir.dt.bfloat16)
                # HW indirect DMA only honors [P, 1] offset APs (one index
                # per partition); multi-index offsets gather garbage.
                nc.gpsimd.indirect_dma_start(
                    out=g[:npar],
                    out_offset=None,
                    in_=table,
                    in_offset=bass.IndirectOffsetOnAxis(
                        ap=idx_t[:npar, c : c + 1], axis=0
                    ),
                )
                gs.append(g)
            gsj.append(gs)
        for j in range(SEGS_PER_EX):
            gs = gsj[j]
            acc = acc_pool.tile([P, H], mybir.dt.bfloat16)
            nc.vector.tensor_tensor(
                out=acc[:npar], in0=gs[0][:npar], in1=gs[1][:npar],
                op=mybir.AluOpType.add,
            )
            for k in range(2, NODES_PER_SEG):
                nc.vector.tensor_tensor(
                    out=acc[:npar], in0=acc[:npar], in1=gs[k][:npar],
                    op=mybir.AluOpType.add,
                )
            nc.scalar.mul(out_t[:npar, j, 0:H], acc[:npar], 1.0 / NODES_PER_SEG)

        rows = slice(b * P, b * P + npar)
        nc.sync.dma_start(out=out[rows, :, :], in_=out_t[:npar])


def _build_nc():
    nc = bacc.Bacc(
        "TRN2",
        target_bir_lowering=False,
        debug=False,
        enable_asserts=False,
        num_devices=N_CORES,
        # 4x the default descriptor carveout: steady state holds ~1280
        # descriptor-pairs in flight (~5 instructions x 256), above the
        # default 16KB scratch's ~1024-descriptor ring -> backpressure
        # stalls inside/between instructions. 64KB removes that ceiling.
        dynamic_dma_scratch_size=65536,
    )
    f32 = mybir.dt.float32
    table = nc.dram_tensor(
        "table", [NUM_ENTITIES, H], mybir.dt.bfloat16, kind="ExternalInput"
    ).ap()
    idx = nc.dram_tensor(
        "idx", [NBLK, P, SEGS_PER_EX * NODES_PER_SEG], mybir.dt.int32,
        kind="ExternalInput",
    ).ap()
    out = nc.dram_tensor(
        "out", [EX_PER_CORE, SEGS_PER_EX, H], f32, kind="ExternalOutput"
    ).ap()
    with tile.TileContext(nc) as tc:
        _emit(tc, table, idx, out)
    nc.compile()
    return nc


def kernel(
    ent_embeds, t_w, t_b, flat_s, node_seg_ids, seg_example, seg_pos, time_vals
):
    import ml_dtypes

    ent_embeds = np.ascontiguousarray(
        np.asarray(ent_embeds, dtype=np.float32).astype(ml_dtypes.bfloat16)
    )
    t_w = np.asarray(t_w, dtype=np.float32)
    t_b = np.asarray(t_b, dtype=np.float32)
    flat_s = np.asarray(flat_s, dtype=np.int32)
    seg_example = np.asarray(seg_example, dtype=np.int32)
    seg_pos = np.asarray(seg_pos, dtype=np.int32)
    time_vals = np.asarray(time_vals, dtype=np.int32)

    if "nc" not in _CACHE:
        _CACHE["nc"] = _build_nc()
    nc = _CACHE["nc"]

    in_maps = []
    for c in range(N_CORES):
        e0 = c * EX_PER_CORE
        fs = flat_s[
            e0 * SEGS_PER_EX * NODES_PER_SEG : (e0 + EX_PER_CORE)
            * SEGS_PER_EX
            * NODES_PER_SEG
        ].reshape(EX_PER_CORE, SEGS_PER_EX * NODES_PER_SEG)
        idx_host = np.zeros((NBLK * P, SEGS_PER_EX * NODES_PER_SEG), np.int32)
        idx_host[:EX_PER_CORE] = fs
        in_maps.append(
            {
                "table": ent_embeds,
                "idx": idx_host.reshape(NBLK, P, SEGS_PER_EX * NODES_PER_SEG),
            }
        )

    trace = os.environ.get("BASSKERNEL_TRACE", "0") == "1"
    kw = {}
    if trace:
        kw = dict(trace=True, tmpdir=os.environ.get("BASSKERNEL_TRACEDIR") or None)
    res = run_bass_kernel_spmd(nc, in_maps, core_ids=list(range(N_CORES)), **kw)
    if trace:
        _CACHE["last_results"] = res
        print(f"[kernel] exec_time_ns={res.exec_time_ns}", file=sys.stderr)

    # Host assembly: time features from a cos LUT over the distinct int t
    # values; pad half is constant; device shards fill the active embeds.
    out = np.empty((N_EXAMPLES, SEQ_LEN, H + T), np.float32)
    pad_vec = np.cos(np.float32(PAD_TIME) * t_w + t_b).astype(np.float32)
    out[:, :, H:] = pad_vec
    out[:, :, :H] = 0.0
    uvals, inv = np.unique(time_vals, return_inverse=True)
    lutv = np.cos(uvals[:, None].astype(np.float32) * t_w + t_b).astype(
        np.float32
    )
    out[seg_example, seg_pos, H:] = lutv[inv]
    for c in range(N_CORES):
        e0 = c * EX_PER_CORE
        out[e0 : e0 + EX_PER_CORE, :SEGS_PER_EX, :H] = np.asarray(
            res.results[c]["out"]
        )
    return out
